# revision 4
# baseline (speedup 1.0000x reference)
"""Trainium2 Bass kernel for nn_BrainInspiredRNN (GRU-like RNN, low-rank recurrent weights).

Strategy (data-parallel over 8 NeuronCores, batch sharded B=4096 -> 512/core):
  - The e2e cost over axon-tunneled cores is dominated by host<->device
    transfer, so I/O is minimized: x is uploaded as fp16 [T,3,BS] per core
    (12.6 MB total) and the readout y = Wout@h + b_out is computed ON DEVICE
    so only yscr [T,2,BS] fp16 (8.4 MB total) comes back -- instead of the
    full hidden-state history (268 MB fp32).
  - Host precomputes fused weight matrices:
      blob32 [64, 133] fp32 : [Vr@Ur.T | Vz@Uz.T | Vn@Un.T | I32 | Wout.T |
                               b_rz | b_hn | b_in]
      blob16 [3, 128] fp16  : [Wir.T | Wiz.T | 0 | Win.T]
  - Device scan, h kept h-major [32, 512] fp32 in SBUF.  Per step:
      psumG[96,512]  = Wfull.T @ h  (+ WiExt.T @ x_t fp16)   (TensorE)
      rz    = sigmoid(psumG[0:64] + b_rz)                    (ScalarE)
      m2    = (psumG[64:96] + b_hn) * r                      (VectorE STT)
      psumN = Win3.T @ x_t (fp16) + I32 @ m2                 (TensorE accumulate)
      n     = tanh(psumN + b_in)                             (ScalarE)
      h'    = n + z * (h - n)                                (VectorE x3)
      py    = WoutT.T @ h'                                   (TensorE, off crit path)
      ysb[.., t] = copy(py) fp16                             (ScalarE cast)
      every CHUNK steps: ysb -> DRAM yscr[t0:t0+CHUNK]       (DMA)
  - Host adds b_out and restores [B,T,2] layout (one fused strided pass).
"""

import os
import sys

import numpy as np

for _p in ("/opt/trn_rl_repo", "/root/.axon_site/_ro/trn_rl_repo"):
    if os.path.isdir(_p) and _p not in sys.path:
        sys.path.insert(0, _p)

import concourse.bacc as bacc
import concourse.bass as bass
import concourse.mybir as mybir
import concourse.tile as tile
from concourse.bass_utils import run_bass_kernel_spmd

B, T, NIN, H, NOUT = 4096, 512, 3, 32, 2
NCORES = 8
BS = B // NCORES          # batch per core
CHUNK = 16                # time steps per x-stage DMA chunk
NSTEP = T
TPAD = ((NSTEP + CHUNK - 1) // CHUNK) * CHUNK
FP32 = mybir.dt.float32
FP16 = mybir.dt.float16

_nc_cache = {}


def _build_program(nsteps=NSTEP):
    key = ("nc", nsteps)
    if key in _nc_cache:
        return _nc_cache[key]

    nc = bacc.Bacc()

    xt_d = nc.declare_dram_parameter("xt", [TPAD, NIN, BS], FP16, isOutput=False)
    h0t_d = nc.declare_dram_parameter("h0t", [H, BS], FP32, isOutput=False)
    blob_d = nc.declare_dram_parameter("blob", [2 * H, 133], FP32, isOutput=False)
    blob16_d = nc.declare_dram_parameter("blob16", [NIN, 128], FP16, isOutput=False)
    yscr_d = nc.declare_dram_parameter("yscr", [T, NOUT, BS], FP16, isOutput=True)

    SIG = mybir.ActivationFunctionType.Sigmoid
    TANH = mybir.ActivationFunctionType.Tanh
    COPY = mybir.ActivationFunctionType.Copy
    MULT = mybir.AluOpType.mult
    ADD = mybir.AluOpType.add
    SUB = mybir.AluOpType.subtract

    with tile.TileContext(nc) as tc:
        with (
            tc.tile_pool(name="const", bufs=1) as cpool,
            tc.tile_pool(name="xstage", bufs=2) as xpool,
            tc.tile_pool(name="ystage", bufs=2) as ypool,
            tc.tile_pool(name="hpool", bufs=3) as hpool,
            tc.tile_pool(name="rzpool", bufs=2) as rzpool,
            tc.tile_pool(name="tmp", bufs=2) as tpool,
            tc.tile_pool(name="psg", bufs=3, space="PSUM") as pgpool,
            tc.tile_pool(name="psn", bufs=3, space="PSUM") as pnpool,
            tc.tile_pool(name="psy", bufs=2, space="PSUM") as pypool,
        ):
            # constants / weights: one fp32 blob + one fp16 blob, sliced
            blob = cpool.tile([2 * H, 133], FP32, tag="blob")
            nc.sync.dma_start(blob[:], blob_d[:])
            blob16 = cpool.tile([NIN, 128], FP16, tag="blob16")
            nc.sync.dma_start(blob16[:], blob16_d[:])
            wf = blob[0:H, 0:96]
            eye = blob[0:H, 96:128]
            woutT = blob[0:H, 128:130]
            brz = blob[0:2 * H, 130:131]
            bhn = blob[0:H, 131:132]
            bin_ = blob[0:H, 132:133]

            h_prev = hpool.tile([H, BS], FP32, tag="h")
            nc.sync.dma_start(h_prev[:], h0t_d[:])

            xs = None
            ysb = None
            for s in range(nsteps):
                toff = s % CHUNK
                if toff == 0:
                    xs = xpool.tile([NIN, CHUNK * BS], FP16, tag="xs")
                    src = xt_d[s:s + CHUNK].rearrange("t c b -> c t b")
                    dst = xs[:, :].rearrange("c (t b) -> c t b", t=CHUNK)
                    nc.sync.dma_start(dst, src)
                    ysb = ypool.tile([NOUT, CHUNK * BS], FP16, tag="ysb")

                xcur = xs[0:NIN, toff * BS:(toff + 1) * BS]

                pg = pgpool.tile([96, BS], FP32, tag="pg")
                nc.tensor.matmul(pg[:], wf, h_prev[:], start=True, stop=False)
                nc.tensor.matmul(pg[:], blob16[0:NIN, 0:96], xcur,
                                 start=False, stop=True)

                pn = pnpool.tile([H, BS], FP32, tag="pn")
                nc.tensor.matmul(pn[:], blob16[0:NIN, 96:128], xcur,
                                 start=True, stop=False)

                rz = rzpool.tile([2 * H, BS], FP32, tag="rz")
                nc.scalar.activation(rz[:], pg[0:64, :], SIG, bias=brz)

                m2 = tpool.tile([H, BS], FP32, tag="m2")
                nc.vector.scalar_tensor_tensor(
                    m2[:], pg[64:96, :], bhn, rz[0:H, :], op0=ADD, op1=MULT)

                nc.tensor.matmul(pn[:], eye, m2[:], start=False, stop=True)

                nn = tpool.tile([H, BS], FP32, tag="nn")
                nc.scalar.activation(nn[:], pn[:], TANH, bias=bin_)

                # dd parked at partitions 32:64 so the zd tensor_tensor sees
                # equal SBUF base partitions (walrus samePartitionsAll rule)
                dd = tpool.tile([2 * H, BS], FP32, tag="dd")
                nc.vector.tensor_tensor(dd[H:2 * H, :], h_prev[:], nn[:], op=SUB)

                zd = tpool.tile([H, BS], FP32, tag="zd")
                nc.vector.tensor_tensor(zd[:], rz[H:2 * H, :], dd[H:2 * H, :],
                                        op=MULT)

                h_new = hpool.tile([H, BS], FP32, tag="h")
                nc.vector.tensor_tensor(h_new[:], nn[:], zd[:], op=ADD)

                # readout (off the recurrence critical path)
                py = pypool.tile([NOUT, BS], FP32, tag="py")
                nc.tensor.matmul(py[:], woutT, h_new[:], start=True, stop=True)
                nc.scalar.activation(ysb[0:NOUT, toff * BS:(toff + 1) * BS],
                                     py[:], COPY)
                if toff == CHUNK - 1:
                    # keep partition dim (c) leading on the SBUF source view:
                    # a partition-reordering rearrange ("-> t c b") on the
                    # source defeats the tile dependency tracker and the DMA
                    # launches before the 16 per-step slice writes complete
                    dst = yscr_d[s - CHUNK + 1:s + 1].rearrange(
                        "t c b -> c t b")
                    src = ysb[:, :].rearrange("c (t b) -> c t b", t=CHUNK)
                    nc.sync.dma_start(dst, src)

                h_prev = h_new

    if not nc.is_finalized():
        nc.finalize()   # Bacc: runs wait-legalization + register allocation
    _nc_cache[key] = nc
    return nc


def _prep_inputs(x, h0, Wir, b_ir, Wiz, b_iz, Win, b_in,
                 Ur, Vr, b_hr, Uz, Vz, b_hz, Un, Vn, b_hn, Wout, b_out):
    f = np.float32
    wfull = np.concatenate(
        [Vr @ Ur.T, Vz @ Uz.T, Vn @ Un.T], axis=1).astype(f)
    eye = np.eye(H, dtype=f)
    blob = np.zeros((2 * H, 133), f)
    blob[0:H, 0:96] = wfull
    blob[0:H, 96:128] = eye
    blob[0:H, 128:130] = Wout.T
    blob[0:2 * H, 130] = np.concatenate([b_ir + b_hr, b_iz + b_hz])
    blob[0:H, 131] = b_hn
    blob[0:H, 132] = b_in

    blob16 = np.zeros((NIN, 128), np.float16)
    blob16[:, 0:H] = Wir.T
    blob16[:, H:2 * H] = Wiz.T
    blob16[:, 96:128] = Win.T

    # xt: [NCORES, TPAD, NIN, BS] (TPAD == T), time-major transposed, fp16
    assert TPAD == T
    x16 = np.asarray(x, dtype=np.float16)
    xt = np.ascontiguousarray(
        x16.reshape(NCORES, BS, T, NIN).transpose(0, 2, 3, 1))
    h0t = np.ascontiguousarray(
        np.asarray(h0, f).reshape(NCORES, BS, H).transpose(0, 2, 1))

    in_maps = []
    for i in range(NCORES):
        in_maps.append({"xt": xt[i], "h0t": h0t[i],
                        "blob": blob, "blob16": blob16})
    return in_maps, np.asarray(b_out, f)


def _run(inputs, trace=False, nsteps=NSTEP, **kw):
    nc = _build_program(nsteps)
    in_maps, b_out = _prep_inputs(**inputs)
    res = run_bass_kernel_spmd(nc, in_maps, list(range(NCORES)),
                               trace=trace, **kw)
    ycat = np.stack([np.asarray(res.results[i]["yscr"])
                     for i in range(NCORES)])          # [8, T, 2, BS] fp16
    yv = ycat.transpose(0, 3, 1, 2)                    # view [8, BS, T, 2]
    y = yv + b_out                                     # fp32, one fused pass
    return np.ascontiguousarray(y.reshape(B, T, NOUT), dtype=np.float32), res


def kernel(**inputs):
    inputs = {k: np.asarray(v) for k, v in inputs.items()}
    y, _ = _run(inputs, trace=False)
    return y


# revision 8
# speedup vs baseline: 1.4426x; 1.4426x over previous
"""Trainium2 Bass kernel for nn_BrainInspiredRNN (GRU-like RNN, low-rank recurrent weights).

Strategy (data-parallel over 8 NeuronCores, batch sharded B=4096 -> 512/core):
  - The e2e cost over axon-tunneled cores is dominated by host<->device
    transfer, so I/O is minimized: x is uploaded as fp16 [T,3,BS] per core
    (12.6 MB total) and the readout y = Wout@h + b_out is computed ON DEVICE
    so only yscr [T,2,BS] fp16 (8.4 MB total) comes back -- instead of the
    full hidden-state history (268 MB fp32).
  - Host precomputes fused weight matrices:
      blob32 [64, 133] fp32 : [Vr@Ur.T | Vz@Uz.T | Vn@Un.T | I32 | Wout.T |
                               b_rz | b_hn | b_in]
      blob16 [3, 128] fp16  : [Wir.T | Wiz.T | 0 | Win.T]
  - Device scan, h kept h-major [32, 512] fp32 in SBUF.  Per step:
      psumG[96,512]  = Wfull.T @ h  (+ WiExt.T @ x_t fp16)   (TensorE)
      rz    = sigmoid(psumG[0:64] + b_rz)                    (ScalarE)
      m2    = (psumG[64:96] + b_hn) * r                      (VectorE STT)
      psumN = Win3.T @ x_t (fp16) + I32 @ m2                 (TensorE accumulate)
      n     = tanh(psumN + b_in)                             (ScalarE)
      h'    = n + z * (h - n)                                (VectorE x3)
      py    = WoutT.T @ h'                                   (TensorE, off crit path)
      ysb[.., t] = copy(py) fp16                             (ScalarE cast)
      every CHUNK steps: ysb -> DRAM yscr[t0:t0+CHUNK]       (DMA)
  - Host adds b_out and restores [B,T,2] layout (one fused strided pass).
"""

import os
import sys

import numpy as np

for _p in ("/opt/trn_rl_repo", "/root/.axon_site/_ro/trn_rl_repo"):
    if os.path.isdir(_p) and _p not in sys.path:
        sys.path.insert(0, _p)

import concourse.bacc as bacc
import concourse.bass as bass
import concourse.mybir as mybir
import concourse.tile as tile
from concourse.bass import ds
from concourse.bass_utils import run_bass_kernel_spmd

B, T, NIN, H, NOUT = 4096, 512, 3, 32, 2
NCORES = 8
BS = B // NCORES          # batch per core
CHUNK = 16                # time steps per x-stage DMA chunk
NSTEP = T
TPAD = ((NSTEP + CHUNK - 1) // CHUNK) * CHUNK
FP32 = mybir.dt.float32
FP16 = mybir.dt.float16

_nc_cache = {}


def _build_program(nsteps=NSTEP):
    key = ("nc", nsteps)
    if key in _nc_cache:
        return _nc_cache[key]

    nc = bacc.Bacc()

    xt_d = nc.declare_dram_parameter("xt", [TPAD, NIN, BS], FP16, isOutput=False)
    h0t_d = nc.declare_dram_parameter("h0t", [H, BS], FP32, isOutput=False)
    blob_d = nc.declare_dram_parameter("blob", [2 * H, 133], FP32, isOutput=False)
    blob16_d = nc.declare_dram_parameter("blob16", [NIN, 128], FP16, isOutput=False)
    yscr_d = nc.declare_dram_parameter("yscr", [T, NOUT, BS], FP16, isOutput=True)

    SIG = mybir.ActivationFunctionType.Sigmoid
    TANH = mybir.ActivationFunctionType.Tanh
    COPY = mybir.ActivationFunctionType.Copy
    MULT = mybir.AluOpType.mult
    ADD = mybir.AluOpType.add
    SUB = mybir.AluOpType.subtract

    assert nsteps % CHUNK == 0
    with tile.TileContext(nc) as tc:
        with (
            tc.tile_pool(name="const", bufs=1) as cpool,
            tc.tile_pool(name="xstage", bufs=1) as xpool,
            tc.tile_pool(name="ystage", bufs=1) as ypool,
            tc.tile_pool(name="hpool", bufs=2) as hpool,
            tc.tile_pool(name="rzpool", bufs=2) as rzpool,
            tc.tile_pool(name="tmp", bufs=2) as tpool,
            tc.tile_pool(name="psg", bufs=3, space="PSUM") as pgpool,
            tc.tile_pool(name="psn", bufs=3, space="PSUM") as pnpool,
            tc.tile_pool(name="psy", bufs=2, space="PSUM") as pypool,
        ):
            # constants / weights: one fp32 blob + one fp16 blob, sliced
            blob = cpool.tile([2 * H, 133], FP32, tag="blob")
            nc.sync.dma_start(blob[:], blob_d[:])
            blob16 = cpool.tile([NIN, 128], FP16, tag="blob16")
            nc.sync.dma_start(blob16[:], blob16_d[:])
            wf = blob[0:H, 0:96]
            eye = blob[0:H, 96:128]
            woutT = blob[0:H, 128:130]
            brz = blob[0:2 * H, 130:131]
            bhn = blob[0:H, 131:132]
            bin_ = blob[0:H, 132:133]

            # h carry: the LAST step of each loop body writes h_new directly
            # into this fixed tile, so each iteration starts by reading it --
            # no reliance on pool-cursor rotation across the back edge (the
            # For_i back-edge drain covers the cross-iteration dependency).
            h0 = cpool.tile([H, BS], FP32, tag="hcarry")
            nc.sync.dma_start(h0[:], h0t_d[:])

            # hardware loop over time chunks: keeps the program ~32x smaller
            # than full unroll (BIR hash + NEFF reload dominate the warm
            # e2e call otherwise)
            with tc.For_i(0, nsteps, CHUNK) as tch:
                h_prev = h0
                xs = xpool.tile([NIN, CHUNK * BS], FP16, tag="xs")
                src = xt_d[ds(tch, CHUNK)].rearrange("t c b -> c t b")
                dst = xs[:, :].rearrange("c (t b) -> c t b", t=CHUNK)
                nc.sync.dma_start(dst, src)
                ysb = ypool.tile([NOUT, CHUNK * BS], FP16, tag="ysb")

                for toff in range(CHUNK):
                    xcur = xs[0:NIN, toff * BS:(toff + 1) * BS]

                    pg = pgpool.tile([96, BS], FP32, tag="pg")
                    nc.tensor.matmul(pg[:], wf, h_prev[:], start=True,
                                     stop=False)
                    nc.tensor.matmul(pg[:], blob16[0:NIN, 0:96], xcur,
                                     start=False, stop=True)

                    pn = pnpool.tile([H, BS], FP32, tag="pn")
                    nc.tensor.matmul(pn[:], blob16[0:NIN, 96:128], xcur,
                                     start=True, stop=False)

                    rz = rzpool.tile([2 * H, BS], FP32, tag="rz")
                    nc.scalar.activation(rz[:], pg[0:64, :], SIG, bias=brz)

                    m2 = tpool.tile([H, BS], FP32, tag="m2")
                    nc.vector.scalar_tensor_tensor(
                        m2[:], pg[64:96, :], bhn, rz[0:H, :], op0=ADD,
                        op1=MULT)

                    nc.tensor.matmul(pn[:], eye, m2[:], start=False, stop=True)

                    nn = tpool.tile([H, BS], FP32, tag="nn")
                    nc.scalar.activation(nn[:], pn[:], TANH, bias=bin_)

                    # dd parked at partitions 32:64 so the zd tensor_tensor
                    # sees equal SBUF base partitions (walrus
                    # samePartitionsAll rule)
                    dd = tpool.tile([2 * H, BS], FP32, tag="dd")
                    nc.vector.tensor_tensor(dd[H:2 * H, :], h_prev[:], nn[:],
                                            op=SUB)

                    zd = tpool.tile([H, BS], FP32, tag="zd")
                    nc.vector.tensor_tensor(zd[:], rz[H:2 * H, :],
                                            dd[H:2 * H, :], op=MULT)

                    if toff == CHUNK - 1:
                        h_new = h0
                    else:
                        h_new = hpool.tile([H, BS], FP32, tag="h")
                    nc.vector.tensor_tensor(h_new[:], nn[:], zd[:], op=ADD)

                    # readout (off the recurrence critical path)
                    py = pypool.tile([NOUT, BS], FP32, tag="py")
                    nc.tensor.matmul(py[:], woutT, h_new[:], start=True,
                                     stop=True)
                    nc.scalar.activation(
                        ysb[0:NOUT, toff * BS:(toff + 1) * BS], py[:], COPY)

                    h_prev = h_new

                # keep partition dim (c) leading on the SBUF source view: a
                # partition-reordering rearrange ("-> t c b") on the source
                # defeats the tile dependency tracker and the DMA launches
                # before the 16 per-step slice writes complete
                dst = yscr_d[ds(tch, CHUNK)].rearrange("t c b -> c t b")
                src = ysb[:, :].rearrange("c (t b) -> c t b", t=CHUNK)
                nc.sync.dma_start(dst, src)

    if not nc.is_finalized():
        nc.finalize()   # Bacc: runs wait-legalization + register allocation
    _nc_cache[key] = nc
    return nc


def _prep_inputs(x, h0, Wir, b_ir, Wiz, b_iz, Win, b_in,
                 Ur, Vr, b_hr, Uz, Vz, b_hz, Un, Vn, b_hn, Wout, b_out):
    f = np.float32
    wfull = np.concatenate(
        [Vr @ Ur.T, Vz @ Uz.T, Vn @ Un.T], axis=1).astype(f)
    eye = np.eye(H, dtype=f)
    blob = np.zeros((2 * H, 133), f)
    blob[0:H, 0:96] = wfull
    blob[0:H, 96:128] = eye
    blob[0:H, 128:130] = Wout.T
    blob[0:2 * H, 130] = np.concatenate([b_ir + b_hr, b_iz + b_hz])
    blob[0:H, 131] = b_hn
    blob[0:H, 132] = b_in

    blob16 = np.zeros((NIN, 128), np.float16)
    blob16[:, 0:H] = Wir.T
    blob16[:, H:2 * H] = Wiz.T
    blob16[:, 96:128] = Win.T

    # xt: [NCORES, TPAD, NIN, BS] (TPAD == T), time-major transposed, fp16
    assert TPAD == T
    x16 = np.asarray(x, dtype=np.float16)
    xt = np.ascontiguousarray(
        x16.reshape(NCORES, BS, T, NIN).transpose(0, 2, 3, 1))
    h0t = np.ascontiguousarray(
        np.asarray(h0, f).reshape(NCORES, BS, H).transpose(0, 2, 1))

    in_maps = []
    for i in range(NCORES):
        in_maps.append({"xt": xt[i], "h0t": h0t[i],
                        "blob": blob, "blob16": blob16})
    return in_maps, np.asarray(b_out, f)


def _run(inputs, trace=False, nsteps=NSTEP, **kw):
    nc = _build_program(nsteps)
    in_maps, b_out = _prep_inputs(**inputs)
    res = run_bass_kernel_spmd(nc, in_maps, list(range(NCORES)),
                               trace=trace, **kw)
    ycat = np.stack([np.asarray(res.results[i]["yscr"])
                     for i in range(NCORES)])          # [8, T, 2, BS] fp16
    yv = ycat.transpose(0, 3, 1, 2)                    # view [8, BS, T, 2]
    y = yv + b_out                                     # fp32, one fused pass
    return np.ascontiguousarray(y.reshape(B, T, NOUT), dtype=np.float32), res


def kernel(**inputs):
    inputs = {k: np.asarray(v) for k, v in inputs.items()}
    y, _ = _run(inputs, trace=False)
    return y
